# revision 1
# baseline (speedup 1.0000x reference)
"""DPCNN (nn_DPCNN) Trainium2 kernel — 8-core data parallel.

Strategy:
  * Host: embedding gather + pad-mask (zero-FLOP LUT), weight reshuffle into
    transposed lhsT tiles, final 5-head select + log-softmax + NLL (tiny).
  * Device (per core, 8 samples each): region conv (E=768 -> C=256), two
    BN(train)+ReLU+conv layers with cross-core AllReduce of per-channel
    sum/sumsq batch stats, then the 8-level maxpool/conv/conv residual
    pyramid down to length 1. Returns features [C] per sample.
  * All matmuls run as float32r (FP22 multiplies, fp32 accumulate in PSUM)
    which streams at 1 cycle/row when the moving dim >= 256. Activations are
    stored in a packed layout: 8 samples separated by a single shared zero
    (or shared BN-pad) column, so every conv tap is one contiguous moving
    operand; junk boundary columns are dropped on PSUM copy-out.

Self-contained: hardcodes shapes from the problem spec.
"""
import numpy as np

import concourse.bass as bass
import concourse.bacc as bacc
import concourse.tile as tile
import concourse.mybir as mybir
from concourse import bass_utils

F32 = mybir.dt.float32
F32R = mybir.dt.float32r
AF = mybir.ActivationFunctionType
ALU = mybir.AluOpType
AX = mybir.AxisListType

NCORES = 8
B, L, E, C = 64, 512, 768, 256
BLOC = B // NCORES                      # 8 samples per core
NCI_E, NCH = E // 128, C // 128         # 6 input chunks, 2 channel chunks
PAD_ID = 1
BN_EPS = 1e-5
NTOT = float(B * L)                     # BN denominator (pads included)
S1 = L - 1                              # 511: sample stride for L=510 stages
W1 = BLOC * S1 + 1                      # 4089: packed width of stride-511 stages
LEVELS = [510, 255, 127, 63, 31, 15, 7, 3]   # pyramid block input lengths

_CACHE = {}


def _sv(base: bass.AP, dims) -> bass.AP:
    """Strided view: keep base's partition dim + offset, replace free dims."""
    ap_list = [list(base.ap[0])] + [[s, c] for (s, c) in dims]
    return bass.AP(base.tensor, base.offset, ap_list)


def _build():
    nc = bacc.Bacc("TRN2", target_bir_lowering=False, debug=False,
                   enable_asserts=True, num_devices=NCORES)

    xe_d = nc.dram_tensor("xe", [NCI_E, BLOC, 128, L], F32R,
                          kind="ExternalInput")
    wr_d = nc.dram_tensor("wr", [128, 3 * NCI_E * NCH * 128], F32R,
                          kind="ExternalInput")
    wc_d = nc.dram_tensor("wc", [128, 3 * NCH * NCH * 128], F32R,
                          kind="ExternalInput")
    vec_d = nc.dram_tensor("vec", [NCH, 128, 6], F32, kind="ExternalInput")
    vec2_d = nc.dram_tensor("vec2", [128, 8], F32, kind="ExternalInput")
    feats_d = nc.dram_tensor("feats", [NCH, 128, BLOC], F32,
                             kind="ExternalOutput")

    with tile.TileContext(nc) as tc:
        _body(nc, tc, xe_d, wr_d, wc_d, vec_d, vec2_d, feats_d)
    nc.compile()
    return nc


def _body(nc, tc, xe_d, wr_d, wc_d, vec_d, vec2_d, feats_d):
    psp = tc.alloc_tile_pool(name="psp", bufs=8, space="PSUM")
    drp = tc.alloc_tile_pool(name="drp", bufs=1, space="DRAM")
    perm = tc.alloc_tile_pool(name="perm", bufs=1)
    bigp = tc.alloc_tile_pool(name="bigp", bufs=1)
    xep = tc.alloc_tile_pool(name="xep", bufs=1)

    # ---- persistent small tiles ----
    wc_t = perm.tile([128, 3 * NCH * NCH * 128], F32R, name="wc_t")
    vec_t = [perm.tile([128, 6], F32, name=f"vec{c2}") for c2 in range(NCH)]
    vec2_t = perm.tile([128, 8], F32, name="vec2_t")
    zeros = perm.tile([128, 16], F32, name="zeros")
    epsT = perm.tile([128, 1], F32, name="epsT")
    for c2 in range(NCH):
        nc.gpsimd.dma_start(out=vec_t[c2][:], in_=vec_d.ap()[c2])
    nc.vector.memset(zeros[:], 0.0)
    nc.vector.memset(epsT[:], BN_EPS)
    warm = perm.tile([128, 1], F32, name="warm")
    nc.scalar.activation(warm[:], epsT[:], AF.Sqrt, bias=epsT[:, 0:1],
                         scale=1.0)

    wr_t = xep.tile([128, 3 * NCI_E * NCH * 128], F32R, name="wr_t")
    xe_t = [[xep.tile([128, L], F32R, name=f"xe{s}_{ci}", tag="xe", bufs=48)
             for ci in range(NCI_E)] for s in range(BLOC)]
    for ci in range(3):
        nc.sync.dma_start(out=xe_t[0][ci][:], in_=xe_d.ap()[ci, 0])
    for ci in range(3, NCI_E):
        nc.gpsimd.dma_start(out=xe_t[0][ci][:], in_=xe_d.ap()[ci, 0])
    for ci in range(NCI_E):
        for k in range(3):
            i = k * NCI_E + ci
            nc.gpsimd.dma_start(out=wr_t[:, i * 256:(i + 1) * 256],
                                in_=wr_d.ap()[:, i * 256:(i + 1) * 256])

    def wr_ap(k, ci, c2):
        i = (k * NCI_E + ci) * NCH + c2
        return wr_t[:, i * 128:(i + 1) * 128]

    def wc_ap(k, ci, c2):
        i = (k * NCH + ci) * NCH + c2
        return wc_t[:, i * 128:(i + 1) * 128]

    def big_tile(name, dtype=F32):
        return bigp.tile([128, 4096], dtype, name=name, tag="big", bufs=4)


    def bn_half_stats(tag, y_c2, c2):
        """Local stats for one chunk -> cin DMA + AllGather (returns cag)."""
        bns = perm.tile([128, BLOC * 6], F32, name=f"bns_{tag}{c2}")
        for s in range(BLOC):
            nc.vector.bn_stats(bns[:, 6 * s:6 * s + 6],
                               y_c2[:, s * S1 + 1: s * S1 + 1 + 510])
        agg = perm.tile([128, 2], F32, name=f"agg_{tag}{c2}")
        nc.vector.bn_aggr(agg[:], bns[:])
        st = perm.tile([128, 2], F32, name=f"sth_{tag}{c2}")
        m2 = perm.tile([128, 1], F32, name=f"m2h_{tag}{c2}")
        nloc = float(BLOC * 510)
        nc.vector.tensor_mul(m2[:], agg[:, 0:1], agg[:, 0:1])
        nc.vector.tensor_add(m2[:], agg[:, 1:2], m2[:])
        nc.scalar.mul(st[:, 0:1], agg[:, 0:1], nloc)
        nc.scalar.mul(st[:, 1:2], m2[:], nloc)
        cin = drp.tile([128, 2], F32, name=f"cinh_{tag}{c2}")
        cag = drp.tile([NCORES, 128, 2], F32, name=f"cagh_{tag}{c2}",
                       addr_space="Shared")
        nc.sync.dma_start(out=cin[:], in_=st[:])
        nc.gpsimd.collective_compute(
            "AllGather", ALU.bypass, replica_groups=[list(range(NCORES))],
            ins=[cin[:].opt()], outs=[cag[:].opt()])
        return cag

    def bn_half_apply(tag, y_c2, c2, lidx, cag):
        """Gather-back + finalize + relu-apply for one chunk."""
        allst = perm.tile([128, NCORES, 2], F32, name=f"allsth_{tag}{c2}")
        nc.sync.dma_start(out=allst[:], in_=cag[:].transpose([1, 0, 2]))
        g2t = perm.tile([128, 2], F32, name=f"gsth_{tag}{c2}")
        nc.vector.tensor_reduce(
            g2t[:], _sv(allst[:, 0, 0:1], [(1, 2), (2, NCORES)]),
            axis=AX.X, op=ALU.add)
        sc = perm.tile([128, 4], F32, name=f"sch_{tag}{c2}")
        nc.scalar.mul(sc[:, 0:1], g2t[:, 0:1], 1.0 / NTOT)   # mean
        nc.scalar.mul(sc[:, 1:2], g2t[:, 1:2], 1.0 / NTOT)   # E[x^2]
        m2b = perm.tile([128, 2], F32, name=f"m2bh_{tag}{c2}")
        nc.vector.tensor_mul(m2b[:, 0:1], sc[:, 0:1], sc[:, 0:1])
        nc.vector.tensor_sub(m2b[:, 1:2], sc[:, 1:2], m2b[:, 0:1])
        std = perm.tile([128, 2], F32, name=f"stdh_{tag}{c2}")
        nc.scalar.activation(std[:, 0:1], m2b[:, 1:2], AF.Sqrt,
                             bias=epsT[:, 0:1], scale=1.0)
        nc.vector.reciprocal(std[:, 1:2], std[:, 0:1])
        nc.vector.tensor_mul(sc[:, 2:3],
                             vec2_t[:, 4 * lidx + c2:4 * lidx + c2 + 1],
                             std[:, 1:2])                    # a
        nc.vector.tensor_mul(sc[:, 1:2], sc[:, 0:1], sc[:, 2:3])
        nc.vector.tensor_sub(
            sc[:, 3:4], vec2_t[:, 4 * lidx + 2 + c2:4 * lidx + 3 + c2],
            sc[:, 1:2])                                      # b
        yn_t = big_tile(f"ynh_{tag}{c2}", F32R)
        nc.scalar.activation(
            _sv(yn_t[:, 0:1], [(S1, BLOC + 1), (1, 1)]),
            _sv(zeros[:, 0:1], [(0, BLOC + 1), (1, 1)]),
            AF.Relu, bias=sc[:, 3:4], scale=1.0)
        for s in range(BLOC):
            nc.scalar.activation(
                yn_t[:, s * S1 + 1: s * S1 + 511],
                y_c2[:, s * S1 + 1: s * S1 + 511],
                AF.Relu, bias=sc[:, 3:4], scale=sc[:, 2:3])
        return yn_t

    def bn_layer(tag, y_t, lidx):
        """BatchNorm(train over full batch via AllReduce) + ReLU.
        y_t: raw conv out, packed stride-511, data at z(s)+1..z(s)+510.
        lidx: 0 (g1/be1) or 1 (g2/be2) selecting vec2 columns.
        Returns new f32r packed tiles (halo cols = relu(beta - mean*a))."""
        bns = [perm.tile([128, BLOC * 6], F32, name=f"bns_{tag}{c2}")
               for c2 in range(NCH)]
        for s in range(BLOC):
            for c2 in range(NCH):
                nc.vector.bn_stats(
                    bns[c2][:, 6 * s:6 * s + 6],
                    y_t[c2][:, s * S1 + 1: s * S1 + 1 + 510])
        # aggs cols: mean0, var0, mean1, var1 -> batched [128,2] math
        aggs = perm.tile([128, 4], F32, name=f"aggs_{tag}")
        for c2 in range(NCH):
            nc.vector.bn_aggr(aggs[:, 2 * c2:2 * c2 + 2], bns[c2][:])
        means = _sv(aggs[:, 0:1], [(2, 2)])
        vars_ = _sv(aggs[:, 1:2], [(2, 2)])
        stats = perm.tile([128, 4], F32, name=f"stats_{tag}")
        m2 = perm.tile([128, 2], F32, name=f"m2_{tag}")
        nloc = float(BLOC * 510)
        nc.vector.tensor_mul(m2[:], means, means)
        nc.vector.tensor_add(m2[:], vars_, m2[:])      # E[x^2] local
        nc.scalar.mul(_sv(stats[:, 0:1], [(2, 2)]), means, nloc)
        nc.scalar.mul(_sv(stats[:, 1:2], [(2, 2)]), m2[:], nloc)

        cin = drp.tile([128, 4], F32, name=f"cin_{tag}")
        cag = drp.tile([NCORES, 128, 4], F32, name=f"cag_{tag}",
                       addr_space="Shared")
        nc.sync.dma_start(out=cin[:], in_=stats[:])
        nc.gpsimd.collective_compute(
            "AllGather", ALU.bypass, replica_groups=[list(range(NCORES))],
            ins=[cin[:].opt()], outs=[cag[:].opt()])
        allst = perm.tile([128, NCORES, 4], F32, name=f"allst_{tag}")
        nc.sync.dma_start(out=allst[:], in_=cag[:].transpose([1, 0, 2]))
        gst = perm.tile([128, 4], F32, name=f"gst_{tag}")
        nc.vector.tensor_reduce(
            gst[:], _sv(allst[:, 0, 0:1], [(1, 4), (4, NCORES)]),
            axis=AX.X, op=ALU.add)

        # batched finalize: a = gamma*rsqrt(var+eps), b = beta - mean*a
        mg = perm.tile([128, 2], F32, name=f"mg_{tag}")
        vg = perm.tile([128, 2], F32, name=f"vg_{tag}")
        ab = perm.tile([128, 4], F32, name=f"ab_{tag}")   # cols: a0,a1,b0,b1
        nc.scalar.mul(mg[:], _sv(gst[:, 0:1], [(2, 2)]), 1.0 / NTOT)
        nc.scalar.mul(vg[:], _sv(gst[:, 1:2], [(2, 2)]), 1.0 / NTOT)
        nc.vector.tensor_mul(ab[:, 0:2], mg[:], mg[:])
        nc.vector.tensor_sub(vg[:], vg[:], ab[:, 0:2])   # var
        nc.scalar.activation(vg[:], vg[:], AF.Sqrt, bias=epsT[:, 0:1],
                             scale=1.0)                  # std (in-place)
        nc.vector.reciprocal(vg[:], vg[:])               # rstd
        nc.vector.tensor_mul(ab[:, 0:2], vec2_t[:, 4 * lidx:4 * lidx + 2],
                             vg[:])                      # a
        nc.vector.tensor_mul(vg[:], mg[:], ab[:, 0:2])   # mean*a
        nc.vector.tensor_sub(ab[:, 2:4],
                             vec2_t[:, 4 * lidx + 2:4 * lidx + 4], vg[:])  # b
        yn = [big_tile(f"yn_{tag}{c2}", F32R) for c2 in range(NCH)]
        for c2 in range(NCH):
            # halo columns z(s) = relu(b), shared between adjacent samples
            nc.scalar.activation(
                _sv(yn[c2][:, 0:1], [(S1, BLOC + 1), (1, 1)]),
                _sv(zeros[:, 0:1], [(0, BLOC + 1), (1, 1)]),
                AF.Relu, bias=ab[:, 2 + c2:3 + c2], scale=1.0)
        for s in range(BLOC):
            for c2 in range(NCH):
                nc.scalar.activation(
                    yn[c2][:, s * S1 + 1: s * S1 + 511],
                    y_t[c2][:, s * S1 + 1: s * S1 + 511],
                    AF.Relu, bias=ab[:, 2 + c2:3 + c2],
                    scale=ab[:, c2:c2 + 1])
        return yn

    def conv_stage(dst_t, src, bias_col, lead=1):
        """3-tap C->C conv over stride-511 packed f32r src tiles; copy-out
        (+bias) into dst data cols (dst data starts at z(s)+lead)."""
        for s in range(BLOC):
            for c2 in range(NCH):
                ps = psp.tile([128, 510], F32, name="ps", tag="ps")
                n = 0
                for ci in range(NCH):
                    for k in range(3):
                        nc.tensor.matmul(
                            ps[:], wc_ap(k, ci, c2),
                            src[ci][:, s * S1 + k: s * S1 + k + 510],
                            start=(n == 0), stop=(n == 5))
                        n += 1
                if c2 == 0:
                    nc.scalar.activation(
                        dst_t[c2][:, s * S1 + lead: s * S1 + lead + 510],
                        ps[:], AF.Identity,
                        bias=vec_t[c2][:, bias_col:bias_col + 1], scale=1.0)
                else:
                    nc.vector.tensor_scalar_add(
                        dst_t[c2][:, s * S1 + lead: s * S1 + lead + 510],
                        ps[:], vec_t[c2][:, bias_col:bias_col + 1])

    # ================= phase 1: region conv =================
    y1_t = [big_tile(f"y1_{c2}") for c2 in range(NCH)]
    for s in range(BLOC):
        if s > 0:
            for ci in range(NCI_E):
                eng = nc.sync if ci < 3 else nc.gpsimd
                eng.dma_start(out=xe_t[s][ci][:], in_=xe_d.ap()[ci, s])
    cag1 = [None, None]
    for c2 in range(NCH):
        for s in range(BLOC):
            ps = psp.tile([128, 510], F32, name="ps", tag="ps")
            n = 0
            for ci in range(NCI_E):
                for k in range(3):
                    nc.tensor.matmul(ps[:], wr_ap(k, ci, c2),
                                     xe_t[s][ci][:, k:k + 510],
                                     start=(n == 0), stop=(n == 17))
                    n += 1
            if c2 == 0:
                nc.scalar.activation(
                    y1_t[c2][:, s * S1 + 1: s * S1 + 511], ps[:],
                    AF.Identity, bias=vec_t[c2][:, 0:1], scale=1.0)
            else:
                nc.vector.tensor_scalar_add(
                    y1_t[c2][:, s * S1 + 1: s * S1 + 511], ps[:],
                    vec_t[c2][:, 0:1])
        # stats + AllGather per chunk: chunk 0's collective hides under
        # chunk 1's matmuls
        cag1[c2] = bn_half_stats("bn1", y1_t[c2], c2)
        if c2 == 0:
            for i in range(3 * NCH):
                nc.gpsimd.dma_start(out=wc_t[:, i * 256:(i + 1) * 256],
                                    in_=wc_d.ap()[:, i * 256:(i + 1) * 256])
            nc.gpsimd.dma_start(out=vec2_t[:], in_=vec2_d.ap())
    xep.release()

    # ================= phase 2: BN1, conv1, BN2, conv2 =================
    y1n = [bn_half_apply("bn1", y1_t[c2], c2, 0, cag1[c2])
           for c2 in range(NCH)]
    y2_t = [big_tile(f"y2_{c2}") for c2 in range(NCH)]
    conv_stage(y2_t, y1n, 1)
    y2n = bn_layer("bn2", y2_t, 1)
    x_t = [big_tile(f"x0_{c2}") for c2 in range(NCH)]
    for c2 in range(NCH):
        nc.vector.memset(x_t[c2][:, 0:2], 0.0)
        nc.vector.memset(_sv(x_t[c2][:, S1 + 1:S1 + 2],
                             [(S1, BLOC), (1, 1)]), 0.0)
    conv_stage(x_t, y2n, 1, lead=2)

    # ================= phase 3: pyramid =================
    smlp = tc.alloc_tile_pool(name="smlp", bufs=1)

    def sml_tile(name, dtype=F32):
        return smlp.tile([128, 2064], dtype, name=name, tag="sml", bufs=10)

    def act_zero(out_ap, free_dims):
        nc.scalar.copy(out_ap, _sv(zeros[:, 0:1], free_dims))

    Sin = S1
    for j, Lin in enumerate(LEVELS):
        assert Sin == Lin + 1
        Lp = (Lin - 2) // 2 + 1
        S = Lp + 1
        G = min(BLOC, max(1, 512 // S))  # samples per matmul group
        if S == 64:
            G = 4                        # level 2: N=256 with 2 groups
        ngr = BLOC // G
        N = max(256, G * S)              # moving-dim per matmul (even)
        Wp = max(BLOC * S + 3, N + 4)    # packed scratch width (+read slack)
        # shifted layout: lead zero col 0; z(m) = 1 + m*S; data(s,h) = s*S+2+h

        px = [sml_tile(f"px{j}_{c2}") for c2 in range(NCH)]
        ra = [sml_tile(f"ra{j}_{c2}", F32R) for c2 in range(NCH)]
        rb = [sml_tile(f"rb{j}_{c2}", F32R) for c2 in range(NCH)]
        if ngr == 1:
            for c2 in range(NCH):
                nc.gpsimd.memset(px[c2][:, 0:Wp], 0.0)
                nc.vector.tensor_reduce(
                    _sv(px[c2][:, 2:3], [(S, BLOC), (1, Lp)]),
                    _sv(x_t[c2][:, 2:3], [(Sin, BLOC), (2, Lp), (1, 3)]),
                    axis=AX.X, op=ALU.max)
                nc.scalar.activation(ra[c2][:, 0:Wp], px[c2][:, 0:Wp],
                                     AF.Relu)
                act_zero(rb[c2][:, 0:Wp], [(0, Wp)])
        else:
            # upfront zero fills (no data deps), then per-group pool/relu so
            # the convs pipeline group by group
            for c2 in range(NCH):
                nc.gpsimd.memset(px[c2][:, 0:2], 0.0)
                nc.gpsimd.memset(_sv(px[c2][:, S + 1:S + 2],
                                     [(S, BLOC), (1, 1)]), 0.0)
                for t in (ra[c2], rb[c2]):
                    act_zero(t[:, 0:2], [(0, 2)])
                    act_zero(_sv(t[:, S + 1:S + 2], [(S, BLOC), (1, 1)]),
                             [(0, BLOC), (1, 1)])
                    if Wp > BLOC * S + 2:
                        act_zero(t[:, BLOC * S + 2:Wp],
                                 [(0, Wp - BLOC * S - 2)])
            for g in range(ngr):
                for c2 in range(NCH):
                    nc.vector.tensor_reduce(
                        _sv(px[c2][:, g * G * S + 2: g * G * S + 3],
                            [(S, G), (1, Lp)]),
                        _sv(x_t[c2][:, g * G * Sin + 2: g * G * Sin + 3],
                            [(Sin, G), (2, Lp), (1, 3)]),
                        axis=AX.X, op=ALU.max)
                    nc.scalar.activation(
                        _sv(ra[c2][:, g * G * S + 2: g * G * S + 3],
                            [(S, G), (1, Lp)]),
                        _sv(px[c2][:, g * G * S + 2: g * G * S + 3],
                            [(S, G), (1, Lp)]),
                        AF.Relu)
        for g in range(ngr):
            for c2 in range(NCH):
                ps = psp.tile([128, N], F32, name="ps", tag="ps")
                n = 0
                for ci in range(NCH):
                    for k in range(3):
                        nc.tensor.matmul(
                            ps[:], wc_ap(k, ci, c2),
                            ra[ci][:, g * G * S + k: g * G * S + k + N],
                            start=(n == 0), stop=(n == 5))
                        n += 1
                # rb = relu(conv_a + b_conv), valid cols only
                nc.scalar.activation(
                    _sv(rb[c2][:, g * G * S + 2: g * G * S + 3],
                        [(S, G), (1, Lp)]),
                    _sv(ps[:, 1:2], [(S, G), (1, Lp)]),
                    AF.Relu, bias=vec_t[c2][:, 1:2], scale=1.0)

        xn = [sml_tile(f"xn{j}_{c2}") for c2 in range(NCH)]
        for c2 in range(NCH):
            nc.gpsimd.memset(xn[c2][:, 0:2], 0.0)
            nc.gpsimd.memset(_sv(xn[c2][:, S + 1:S + 2],
                                 [(S, BLOC), (1, 1)]), 0.0)
        for g in range(ngr):
            for c2 in range(NCH):
                ps = psp.tile([128, N], F32, name="ps", tag="ps")
                n = 0
                for ci in range(NCH):
                    for k in range(3):
                        nc.tensor.matmul(
                            ps[:], wc_ap(k, ci, c2),
                            rb[ci][:, g * G * S + k: g * G * S + k + N],
                            start=(n == 0), stop=(n == 5))
                        n += 1
                # x_next = (conv_b + b_conv) + px   (residual)
                nc.vector.scalar_tensor_tensor(
                    _sv(xn[c2][:, g * G * S + 2: g * G * S + 3],
                        [(S, G), (1, Lp)]),
                    _sv(ps[:, 1:2], [(S, G), (1, Lp)]),
                    vec_t[c2][:, 1:2],
                    _sv(px[c2][:, g * G * S + 2: g * G * S + 3],
                        [(S, G), (1, Lp)]),
                    op0=ALU.add, op1=ALU.add)
        x_t = xn
        Sin = S

    # features = x[:, :, 0] per sample  (final packed layout stride 2)
    for c2 in range(NCH):
        nc.sync.dma_start(out=feats_d.ap()[c2],
                          in_=_sv(x_t[c2][:, 2:3], [(2, BLOC), (1, 1)]))
    smlp.release()
    bigp.release()
    perm.release()
    drp.release()
    psp.release()


def _host_inputs(input_ids, emb_table, w_region, b_region, w_conv, b_conv,
                 g1, be1, g2, be2):
    ids = np.asarray(input_ids)
    emb = np.asarray(emb_table)[ids]                     # [B, L, E]
    emb = emb * (ids != PAD_ID)[..., None].astype(np.float32)
    # -> [NCORES, NCI_E, BLOC, 128, L]
    xe = emb.reshape(NCORES, BLOC, L, NCI_E, 128)
    xe = np.ascontiguousarray(xe.transpose(0, 3, 1, 4, 2), dtype=np.float32)

    def pack_lhsT(w, nci):
        # w: [C, nci*128, 3] -> [128, 3*nci*NCH*128]; tile (k,ci,c2) is
        # lhsT[p_in, p_out] = w[c2*128+p_out, ci*128+p_in, k]
        out = np.empty((128, 3 * nci * NCH * 128), np.float32)
        i = 0
        for k in range(3):
            for ci in range(nci):
                for c2 in range(NCH):
                    out[:, i * 128:(i + 1) * 128] = \
                        w[c2 * 128:(c2 + 1) * 128,
                          ci * 128:(ci + 1) * 128, k].T
                    i += 1
        return out

    wr = pack_lhsT(np.asarray(w_region), NCI_E)
    wc = pack_lhsT(np.asarray(w_conv), NCH)
    vec = np.stack([np.asarray(v, np.float32).reshape(NCH, 128)
                    for v in (b_region, b_conv, g1, be1, g2, be2)],
                   axis=-1)                              # [NCH, 128, 6]
    # vec2: [128, 8] cols = g1(c2=0), g1(1), be1(0), be1(1), g2.., be2..
    vec2 = np.stack([np.asarray(v, np.float32).reshape(NCH, 128)[c2]
                     for v in (g1, be1, g2, be2) for c2 in range(NCH)],
                    axis=-1)                             # [128, 8]
    return xe, wr, wc, np.ascontiguousarray(vec), np.ascontiguousarray(vec2)


def _head_loss(features, groups, labels, w_heads, b_heads):
    groups = np.asarray(groups)
    labels = np.asarray(labels)
    w_heads = np.asarray(w_heads)
    b_heads = np.asarray(b_heads)
    logits_all = np.einsum('bd,kdc->bkc', features, w_heads) + b_heads[None]
    idx = np.clip(np.argmax(groups, axis=-1), 0, 4)
    logits = logits_all[np.arange(len(idx)), idx]
    m = logits.max(axis=-1, keepdims=True)
    z = logits - m
    logp = z - np.log(np.exp(z).sum(axis=-1, keepdims=True))
    return np.array(-np.mean(logp[np.arange(len(labels)), labels]),
                    dtype=np.float32)


def _features_from_results(results):
    feats = np.empty((B, C), np.float32)
    for c in range(NCORES):
        f = results[c]["feats"]                          # [NCH, 128, BLOC]
        feats[c * BLOC:(c + 1) * BLOC] = \
            f.transpose(2, 0, 1).reshape(BLOC, C)
    return feats


def kernel(input_ids, groups, labels, emb_table, w_region, b_region,
           w_conv, b_conv, g1, be1, g2, be2, w_heads, b_heads,
           _run_kwargs=None):
    if "nc" not in _CACHE:
        _CACHE["nc"] = _build()
    nc = _CACHE["nc"]

    xe, wr, wc, vec, vec2 = _host_inputs(
        input_ids, emb_table, w_region, b_region, w_conv, b_conv,
        g1, be1, g2, be2)
    in_maps = [{"xe": np.ascontiguousarray(xe[c]), "wr": wr, "wc": wc,
                "vec": vec, "vec2": vec2} for c in range(NCORES)]
    res = bass_utils.run_bass_kernel_spmd(
        nc, in_maps, core_ids=list(range(NCORES)), **(_run_kwargs or {}))
    _CACHE["last_result"] = res
    feats = _features_from_results(res.results)
    _CACHE["features"] = feats
    return _head_loss(feats, groups, labels, w_heads, b_heads)



# revision 5
# speedup vs baseline: 28705.6717x; 28705.6717x over previous
"""DPCNN (nn_DPCNN) Trainium2 kernel — 8-core data parallel.

Strategy:
  * Host: embedding gather + pad-mask (zero-FLOP LUT), weight reshuffle into
    transposed lhsT tiles, final 5-head select + log-softmax + NLL (tiny).
  * Device (per core, 8 samples each): region conv (E=768 -> C=256), two
    BN(train)+ReLU+conv layers with cross-core AllReduce of per-channel
    sum/sumsq batch stats, then the 8-level maxpool/conv/conv residual
    pyramid down to length 1. Returns features [C] per sample.
  * All matmuls run as float32r (FP22 multiplies, fp32 accumulate in PSUM)
    which streams at 1 cycle/row when the moving dim >= 256. Activations are
    stored in a packed layout: 8 samples separated by a single shared zero
    (or shared BN-pad) column, so every conv tap is one contiguous moving
    operand; junk boundary columns are dropped on PSUM copy-out.

Self-contained: hardcodes shapes from the problem spec.
"""
import numpy as np

import concourse.bass as bass
import concourse.bacc as bacc
import concourse.tile as tile
import concourse.mybir as mybir
from concourse import bass_utils

F32 = mybir.dt.float32
F32R = mybir.dt.float32r
AF = mybir.ActivationFunctionType
ALU = mybir.AluOpType
AX = mybir.AxisListType

NCORES = 8
B, L, E, C = 64, 512, 768, 256
BLOC = B // NCORES                      # 8 samples per core
NCI_E, NCH = E // 128, C // 128         # 6 input chunks, 2 channel chunks
PAD_ID = 1
BN_EPS = 1e-5
NTOT = float(B * L)                     # BN denominator (pads included)
S1 = L - 1                              # 511: sample stride for L=510 stages
W1 = BLOC * S1 + 1                      # 4089: packed width of stride-511 stages
LEVELS = [510, 255, 127, 63, 31, 15, 7, 3]   # pyramid block input lengths

_CACHE = {}


def _sv(base: bass.AP, dims) -> bass.AP:
    """Strided view: keep base's partition dim + offset, replace free dims."""
    ap_list = [list(base.ap[0])] + [[s, c] for (s, c) in dims]
    return bass.AP(base.tensor, base.offset, ap_list)


def _build(reps=1):
    nc = bacc.Bacc("TRN2", target_bir_lowering=False, debug=False,
                   enable_asserts=True, num_devices=NCORES)

    xe_d = nc.dram_tensor("xe", [NCI_E, BLOC, 128, L], F32R,
                          kind="ExternalInput")
    wr_d = nc.dram_tensor("wr", [128, 3 * NCI_E * NCH * 128], F32R,
                          kind="ExternalInput")
    wc_d = nc.dram_tensor("wc", [128, 3 * NCH * NCH * 128], F32R,
                          kind="ExternalInput")
    vec_d = nc.dram_tensor("vec", [NCH, 128, 6], F32, kind="ExternalInput")
    vec2_d = nc.dram_tensor("vec2", [128, 8], F32, kind="ExternalInput")
    feats_d = nc.dram_tensor("feats", [NCH, 128, BLOC], F32,
                             kind="ExternalOutput")

    with tile.TileContext(nc) as tc:
        for _ in range(reps):
            _body(nc, tc, xe_d, wr_d, wc_d, vec_d, vec2_d, feats_d)
    nc.compile()
    return nc


def _body(nc, tc, xe_d, wr_d, wc_d, vec_d, vec2_d, feats_d):
    psp = tc.alloc_tile_pool(name="psp", bufs=8, space="PSUM")
    drp = tc.alloc_tile_pool(name="drp", bufs=1, space="DRAM")
    perm = tc.alloc_tile_pool(name="perm", bufs=1)
    bigp = tc.alloc_tile_pool(name="bigp", bufs=1)
    xep = tc.alloc_tile_pool(name="xep", bufs=1)

    # ---- persistent small tiles ----
    wc_t = perm.tile([128, 3 * NCH * NCH * 128], F32R, name="wc_t")
    vec_t = [perm.tile([128, 6], F32, name=f"vec{c2}") for c2 in range(NCH)]
    vec2_t = perm.tile([128, 8], F32, name="vec2_t")
    zeros = perm.tile([128, 16], F32, name="zeros")
    epsT = perm.tile([128, 1], F32, name="epsT")
    for c2 in range(NCH):
        nc.gpsimd.dma_start(out=vec_t[c2][:], in_=vec_d.ap()[c2])
    nc.vector.memset(zeros[:], 0.0)
    nc.vector.memset(epsT[:], BN_EPS)
    warm = perm.tile([128, 1], F32, name="warm")
    nc.scalar.activation(warm[:], epsT[:], AF.Sqrt, bias=epsT[:, 0:1],
                         scale=1.0)

    wr_t = xep.tile([128, 3 * NCI_E * NCH * 128], F32R, name="wr_t")
    xe_t = [[xep.tile([128, L], F32R, name=f"xe{s}_{ci}", tag="xe", bufs=48)
             for ci in range(NCI_E)] for s in range(BLOC)]
    for ci in range(3):
        nc.sync.dma_start(out=xe_t[0][ci][:], in_=xe_d.ap()[ci, 0])
    for ci in range(3, NCI_E):
        nc.gpsimd.dma_start(out=xe_t[0][ci][:], in_=xe_d.ap()[ci, 0])
    for ci in range(NCI_E):
        for k in range(3):
            i = k * NCI_E + ci
            nc.gpsimd.dma_start(out=wr_t[:, i * 256:(i + 1) * 256],
                                in_=wr_d.ap()[:, i * 256:(i + 1) * 256])

    def wr_ap(k, ci, c2):
        i = (k * NCI_E + ci) * NCH + c2
        return wr_t[:, i * 128:(i + 1) * 128]

    def wc_ap(k, ci, c2):
        i = (k * NCH + ci) * NCH + c2
        return wc_t[:, i * 128:(i + 1) * 128]

    def big_tile(name, dtype=F32):
        return bigp.tile([128, 4096], dtype, name=name, tag="big", bufs=4)


    def bn_half_stats(tag, y_c2, c2):
        """Local stats for one chunk -> cin DMA + AllGather (returns cag)."""
        bns = perm.tile([128, BLOC * 6], F32, name=f"bns_{tag}{c2}")
        for s in range(BLOC):
            nc.vector.bn_stats(bns[:, 6 * s:6 * s + 6],
                               y_c2[:, s * S1 + 1: s * S1 + 1 + 510])
        agg = perm.tile([128, 2], F32, name=f"agg_{tag}{c2}")
        nc.vector.bn_aggr(agg[:], bns[:])
        st = perm.tile([128, 2], F32, name=f"sth_{tag}{c2}")
        m2 = perm.tile([128, 1], F32, name=f"m2h_{tag}{c2}")
        nloc = float(BLOC * 510)
        nc.vector.tensor_mul(m2[:], agg[:, 0:1], agg[:, 0:1])
        nc.vector.tensor_add(m2[:], agg[:, 1:2], m2[:])
        nc.scalar.mul(st[:, 0:1], agg[:, 0:1], nloc)
        nc.scalar.mul(st[:, 1:2], m2[:], nloc)
        cin = drp.tile([128, 2], F32, name=f"cinh_{tag}{c2}")
        cag = drp.tile([NCORES, 128, 2], F32, name=f"cagh_{tag}{c2}",
                       addr_space="Shared")
        nc.sync.dma_start(out=cin[:], in_=st[:])
        nc.gpsimd.collective_compute(
            "AllGather", ALU.bypass, replica_groups=[list(range(NCORES))],
            ins=[cin[:].opt()], outs=[cag[:].opt()])
        return cag

    def bn_half_apply(tag, y_c2, c2, lidx, cag):
        """Gather-back + finalize + relu-apply for one chunk."""
        allst = perm.tile([128, NCORES, 2], F32, name=f"allsth_{tag}{c2}")
        nc.sync.dma_start(out=allst[:], in_=cag[:].transpose([1, 0, 2]))
        g2t = perm.tile([128, 2], F32, name=f"gsth_{tag}{c2}")
        nc.vector.tensor_reduce(
            g2t[:], _sv(allst[:, 0, 0:1], [(1, 2), (2, NCORES)]),
            axis=AX.X, op=ALU.add)
        sc = perm.tile([128, 4], F32, name=f"sch_{tag}{c2}")
        nc.scalar.mul(sc[:, 0:1], g2t[:, 0:1], 1.0 / NTOT)   # mean
        nc.scalar.mul(sc[:, 1:2], g2t[:, 1:2], 1.0 / NTOT)   # E[x^2]
        m2b = perm.tile([128, 2], F32, name=f"m2bh_{tag}{c2}")
        nc.vector.tensor_mul(m2b[:, 0:1], sc[:, 0:1], sc[:, 0:1])
        nc.vector.tensor_sub(m2b[:, 1:2], sc[:, 1:2], m2b[:, 0:1])
        std = perm.tile([128, 2], F32, name=f"stdh_{tag}{c2}")
        nc.scalar.activation(std[:, 0:1], m2b[:, 1:2], AF.Sqrt,
                             bias=epsT[:, 0:1], scale=1.0)
        nc.vector.reciprocal(std[:, 1:2], std[:, 0:1])
        nc.vector.tensor_mul(sc[:, 2:3],
                             vec2_t[:, 4 * lidx + c2:4 * lidx + c2 + 1],
                             std[:, 1:2])                    # a
        nc.vector.tensor_mul(sc[:, 1:2], sc[:, 0:1], sc[:, 2:3])
        nc.vector.tensor_sub(
            sc[:, 3:4], vec2_t[:, 4 * lidx + 2 + c2:4 * lidx + 3 + c2],
            sc[:, 1:2])                                      # b
        yn_t = big_tile(f"ynh_{tag}{c2}", F32R)
        nc.scalar.activation(
            _sv(yn_t[:, 0:1], [(S1, BLOC + 1), (1, 1)]),
            _sv(zeros[:, 0:1], [(0, BLOC + 1), (1, 1)]),
            AF.Relu, bias=sc[:, 3:4], scale=1.0)
        for s in range(BLOC):
            nc.scalar.activation(
                yn_t[:, s * S1 + 1: s * S1 + 511],
                y_c2[:, s * S1 + 1: s * S1 + 511],
                AF.Relu, bias=sc[:, 3:4], scale=sc[:, 2:3])
        return yn_t

    def bn_layer(tag, y_t, lidx):
        """BatchNorm(train over full batch via AllReduce) + ReLU.
        y_t: raw conv out, packed stride-511, data at z(s)+1..z(s)+510.
        lidx: 0 (g1/be1) or 1 (g2/be2) selecting vec2 columns.
        Returns new f32r packed tiles (halo cols = relu(beta - mean*a))."""
        bns = [perm.tile([128, BLOC * 6], F32, name=f"bns_{tag}{c2}")
               for c2 in range(NCH)]
        for s in range(BLOC):
            for c2 in range(NCH):
                nc.vector.bn_stats(
                    bns[c2][:, 6 * s:6 * s + 6],
                    y_t[c2][:, s * S1 + 1: s * S1 + 1 + 510])
        # aggs cols: mean0, var0, mean1, var1 -> batched [128,2] math
        aggs = perm.tile([128, 4], F32, name=f"aggs_{tag}")
        for c2 in range(NCH):
            nc.vector.bn_aggr(aggs[:, 2 * c2:2 * c2 + 2], bns[c2][:])
        means = _sv(aggs[:, 0:1], [(2, 2)])
        vars_ = _sv(aggs[:, 1:2], [(2, 2)])
        stats = perm.tile([128, 4], F32, name=f"stats_{tag}")
        m2 = perm.tile([128, 2], F32, name=f"m2_{tag}")
        nloc = float(BLOC * 510)
        nc.vector.tensor_mul(m2[:], means, means)
        nc.vector.tensor_add(m2[:], vars_, m2[:])      # E[x^2] local
        nc.scalar.mul(_sv(stats[:, 0:1], [(2, 2)]), means, nloc)
        nc.scalar.mul(_sv(stats[:, 1:2], [(2, 2)]), m2[:], nloc)

        cin = drp.tile([128, 4], F32, name=f"cin_{tag}")
        cag = drp.tile([NCORES, 128, 4], F32, name=f"cag_{tag}",
                       addr_space="Shared")
        nc.sync.dma_start(out=cin[:], in_=stats[:])
        nc.gpsimd.collective_compute(
            "AllGather", ALU.bypass, replica_groups=[list(range(NCORES))],
            ins=[cin[:].opt()], outs=[cag[:].opt()])
        allst = perm.tile([128, NCORES, 4], F32, name=f"allst_{tag}")
        nc.sync.dma_start(out=allst[:], in_=cag[:].transpose([1, 0, 2]))
        gst = perm.tile([128, 4], F32, name=f"gst_{tag}")
        nc.vector.tensor_reduce(
            gst[:], _sv(allst[:, 0, 0:1], [(1, 4), (4, NCORES)]),
            axis=AX.X, op=ALU.add)

        # batched finalize: a = gamma*rsqrt(var+eps), b = beta - mean*a
        mg = perm.tile([128, 2], F32, name=f"mg_{tag}")
        vg = perm.tile([128, 2], F32, name=f"vg_{tag}")
        ab = perm.tile([128, 4], F32, name=f"ab_{tag}")   # cols: a0,a1,b0,b1
        nc.scalar.mul(mg[:], _sv(gst[:, 0:1], [(2, 2)]), 1.0 / NTOT)
        nc.scalar.mul(vg[:], _sv(gst[:, 1:2], [(2, 2)]), 1.0 / NTOT)
        nc.vector.tensor_mul(ab[:, 0:2], mg[:], mg[:])
        nc.vector.tensor_sub(vg[:], vg[:], ab[:, 0:2])   # var
        nc.scalar.activation(vg[:], vg[:], AF.Sqrt, bias=epsT[:, 0:1],
                             scale=1.0)                  # std (in-place)
        nc.vector.reciprocal(vg[:], vg[:])               # rstd
        nc.vector.tensor_mul(ab[:, 0:2], vec2_t[:, 4 * lidx:4 * lidx + 2],
                             vg[:])                      # a
        nc.vector.tensor_mul(vg[:], mg[:], ab[:, 0:2])   # mean*a
        nc.vector.tensor_sub(ab[:, 2:4],
                             vec2_t[:, 4 * lidx + 2:4 * lidx + 4], vg[:])  # b
        yn = [big_tile(f"yn_{tag}{c2}", F32R) for c2 in range(NCH)]
        for c2 in range(NCH):
            # halo columns z(s) = relu(b), shared between adjacent samples
            nc.scalar.activation(
                _sv(yn[c2][:, 0:1], [(S1, BLOC + 1), (1, 1)]),
                _sv(zeros[:, 0:1], [(0, BLOC + 1), (1, 1)]),
                AF.Relu, bias=ab[:, 2 + c2:3 + c2], scale=1.0)
        for s in range(BLOC):
            for c2 in range(NCH):
                nc.scalar.activation(
                    yn[c2][:, s * S1 + 1: s * S1 + 511],
                    y_t[c2][:, s * S1 + 1: s * S1 + 511],
                    AF.Relu, bias=ab[:, 2 + c2:3 + c2],
                    scale=ab[:, c2:c2 + 1])
        return yn

    def conv_stage(dst_t, src, bias_col, lead=1, stats_tag=None):
        """3-tap C->C conv over stride-511 packed f32r src tiles; copy-out
        (+bias) into dst data cols (dst data starts at z(s)+lead).
        stats_tag: c2-outer order + per-chunk BN stats/AllGather right after
        each chunk finishes, so chunk 0's collective hides under chunk 1's
        matmuls (same trick phase 1 uses for BN1). Returns cag list."""
        cags = [None, None] if stats_tag else None
        for c2 in range(NCH):
            for s in range(BLOC):
                ps = psp.tile([128, 510], F32, name="ps", tag="ps")
                n = 0
                for ci in range(NCH):
                    for k in range(3):
                        nc.tensor.matmul(
                            ps[:], wc_ap(k, ci, c2),
                            src[ci][:, s * S1 + k: s * S1 + k + 510],
                            start=(n == 0), stop=(n == 5))
                        n += 1
                if c2 == 0:
                    nc.scalar.activation(
                        dst_t[c2][:, s * S1 + lead: s * S1 + lead + 510],
                        ps[:], AF.Identity,
                        bias=vec_t[c2][:, bias_col:bias_col + 1], scale=1.0)
                else:
                    nc.vector.tensor_scalar_add(
                        dst_t[c2][:, s * S1 + lead: s * S1 + lead + 510],
                        ps[:], vec_t[c2][:, bias_col:bias_col + 1])
            if stats_tag:
                cags[c2] = bn_half_stats(stats_tag, dst_t[c2], c2)
        return cags

    # ================= phase 1: region conv =================
    y1_t = [big_tile(f"y1_{c2}") for c2 in range(NCH)]
    for s in range(BLOC):
        if s > 0:
            for ci in range(NCI_E):
                eng = nc.sync if ci < 3 else nc.gpsimd
                eng.dma_start(out=xe_t[s][ci][:], in_=xe_d.ap()[ci, s])
    cag1 = [None, None]
    for c2 in range(NCH):
        for s in range(BLOC):
            ps = psp.tile([128, 510], F32, name="ps", tag="ps")
            n = 0
            for ci in range(NCI_E):
                for k in range(3):
                    nc.tensor.matmul(ps[:], wr_ap(k, ci, c2),
                                     xe_t[s][ci][:, k:k + 510],
                                     start=(n == 0), stop=(n == 17))
                    n += 1
            if c2 == 0:
                nc.scalar.activation(
                    y1_t[c2][:, s * S1 + 1: s * S1 + 511], ps[:],
                    AF.Identity, bias=vec_t[c2][:, 0:1], scale=1.0)
            else:
                nc.vector.tensor_scalar_add(
                    y1_t[c2][:, s * S1 + 1: s * S1 + 511], ps[:],
                    vec_t[c2][:, 0:1])
        # stats + AllGather per chunk: chunk 0's collective hides under
        # chunk 1's matmuls
        cag1[c2] = bn_half_stats("bn1", y1_t[c2], c2)
        if c2 == 0:
            for i in range(3 * NCH):
                nc.gpsimd.dma_start(out=wc_t[:, i * 256:(i + 1) * 256],
                                    in_=wc_d.ap()[:, i * 256:(i + 1) * 256])
            nc.gpsimd.dma_start(out=vec2_t[:], in_=vec2_d.ap())
    xep.release()

    # ================= phase 2: BN1, conv1, BN2, conv2 =================
    y1n = [bn_half_apply("bn1", y1_t[c2], c2, 0, cag1[c2])
           for c2 in range(NCH)]
    y2_t = [big_tile(f"y2_{c2}") for c2 in range(NCH)]
    cag2 = conv_stage(y2_t, y1n, 1, stats_tag="bn2")
    y2n = [bn_half_apply("bn2", y2_t[c2], c2, 1, cag2[c2])
           for c2 in range(NCH)]
    x_t = [big_tile(f"x0_{c2}") for c2 in range(NCH)]
    for c2 in range(NCH):
        nc.vector.memset(x_t[c2][:, 0:2], 0.0)
        nc.vector.memset(_sv(x_t[c2][:, S1 + 1:S1 + 2],
                             [(S1, BLOC), (1, 1)]), 0.0)
    # conv2 c2=0 with split accumulation: the ci=0 tap matmuls depend only
    # on y2n[0] (whose stats collective hid under conv1 c2=1), so they run
    # while chunk 1's AllGather is still in flight; each PSUM bank is then
    # finished with the ci=1 taps once y2n[1] lands.
    ps0 = []
    for s in range(BLOC):
        ps = psp.tile([128, 510], F32, name="ps", tag="ps")
        for k in range(3):
            nc.tensor.matmul(ps[:], wc_ap(k, 0, 0),
                             y2n[0][:, s * S1 + k: s * S1 + k + 510],
                             start=(k == 0), stop=False)
        ps0.append(ps)
    for s in range(BLOC):
        for k in range(3):
            nc.tensor.matmul(ps0[s][:], wc_ap(k, 1, 0),
                             y2n[1][:, s * S1 + k: s * S1 + k + 510],
                             start=False, stop=(k == 2))
        nc.scalar.activation(x_t[0][:, s * S1 + 2: s * S1 + 2 + 510],
                             ps0[s][:], AF.Identity,
                             bias=vec_t[0][:, 1:2], scale=1.0)
    for s in range(BLOC):
        ps = psp.tile([128, 510], F32, name="ps", tag="ps")
        n = 0
        for ci in range(NCH):
            for k in range(3):
                nc.tensor.matmul(ps[:], wc_ap(k, ci, 1),
                                 y2n[ci][:, s * S1 + k: s * S1 + k + 510],
                                 start=(n == 0), stop=(n == 5))
                n += 1
        nc.vector.tensor_scalar_add(x_t[1][:, s * S1 + 2: s * S1 + 2 + 510],
                                    ps[:], vec_t[1][:, 1:2])

    # ================= phase 3: pyramid =================
    smlp = tc.alloc_tile_pool(name="smlp", bufs=1)

    def sml_tile(name, dtype=F32):
        return smlp.tile([128, 2064], dtype, name=name, tag="sml", bufs=10)

    def act_zero(out_ap, free_dims):
        nc.scalar.copy(out_ap, _sv(zeros[:, 0:1], free_dims))

    Sin = S1
    for j, Lin in enumerate(LEVELS):
        assert Sin == Lin + 1
        Lp = (Lin - 2) // 2 + 1
        S = Lp + 1
        G = min(BLOC, max(1, 512 // S))  # samples per matmul group
        if S == 64:
            G = 4                        # level 2: N=256 with 2 groups
        ngr = BLOC // G
        N = max(256, G * S)              # moving-dim per matmul (even)
        Wp = max(BLOC * S + 3, N + 4)    # packed scratch width (+read slack)
        # shifted layout: lead zero col 0; z(m) = 1 + m*S; data(s,h) = s*S+2+h

        px = [sml_tile(f"px{j}_{c2}") for c2 in range(NCH)]
        ra = [sml_tile(f"ra{j}_{c2}", F32R) for c2 in range(NCH)]
        rb = [sml_tile(f"rb{j}_{c2}", F32R) for c2 in range(NCH)]
        if ngr == 1:
            for c2 in range(NCH):
                nc.gpsimd.memset(px[c2][:, 0:Wp], 0.0)
                nc.vector.tensor_reduce(
                    _sv(px[c2][:, 2:3], [(S, BLOC), (1, Lp)]),
                    _sv(x_t[c2][:, 2:3], [(Sin, BLOC), (2, Lp), (1, 3)]),
                    axis=AX.X, op=ALU.max)
                nc.scalar.activation(ra[c2][:, 0:Wp], px[c2][:, 0:Wp],
                                     AF.Relu)
                act_zero(rb[c2][:, 0:Wp], [(0, Wp)])
        else:
            # upfront zero fills (no data deps), then per-group pool/relu so
            # the convs pipeline group by group
            for c2 in range(NCH):
                nc.gpsimd.memset(px[c2][:, 0:2], 0.0)
                nc.gpsimd.memset(_sv(px[c2][:, S + 1:S + 2],
                                     [(S, BLOC), (1, 1)]), 0.0)
                for t in (ra[c2], rb[c2]):
                    act_zero(t[:, 0:2], [(0, 2)])
                    act_zero(_sv(t[:, S + 1:S + 2], [(S, BLOC), (1, 1)]),
                             [(0, BLOC), (1, 1)])
                    if Wp > BLOC * S + 2:
                        act_zero(t[:, BLOC * S + 2:Wp],
                                 [(0, Wp - BLOC * S - 2)])
            for g in range(ngr):
                for c2 in range(NCH):
                    nc.vector.tensor_reduce(
                        _sv(px[c2][:, g * G * S + 2: g * G * S + 3],
                            [(S, G), (1, Lp)]),
                        _sv(x_t[c2][:, g * G * Sin + 2: g * G * Sin + 3],
                            [(Sin, G), (2, Lp), (1, 3)]),
                        axis=AX.X, op=ALU.max)
                    nc.scalar.activation(
                        _sv(ra[c2][:, g * G * S + 2: g * G * S + 3],
                            [(S, G), (1, Lp)]),
                        _sv(px[c2][:, g * G * S + 2: g * G * S + 3],
                            [(S, G), (1, Lp)]),
                        AF.Relu)
        for g in range(ngr):
            for c2 in range(NCH):
                ps = psp.tile([128, N], F32, name="ps", tag="ps")
                n = 0
                for ci in range(NCH):
                    for k in range(3):
                        nc.tensor.matmul(
                            ps[:], wc_ap(k, ci, c2),
                            ra[ci][:, g * G * S + k: g * G * S + k + N],
                            start=(n == 0), stop=(n == 5))
                        n += 1
                # rb = relu(conv_a + b_conv), valid cols only
                nc.scalar.activation(
                    _sv(rb[c2][:, g * G * S + 2: g * G * S + 3],
                        [(S, G), (1, Lp)]),
                    _sv(ps[:, 1:2], [(S, G), (1, Lp)]),
                    AF.Relu, bias=vec_t[c2][:, 1:2], scale=1.0)

        xn = [sml_tile(f"xn{j}_{c2}") for c2 in range(NCH)]
        for c2 in range(NCH):
            nc.gpsimd.memset(xn[c2][:, 0:2], 0.0)
            nc.gpsimd.memset(_sv(xn[c2][:, S + 1:S + 2],
                                 [(S, BLOC), (1, 1)]), 0.0)
        for g in range(ngr):
            for c2 in range(NCH):
                ps = psp.tile([128, N], F32, name="ps", tag="ps")
                n = 0
                for ci in range(NCH):
                    for k in range(3):
                        nc.tensor.matmul(
                            ps[:], wc_ap(k, ci, c2),
                            rb[ci][:, g * G * S + k: g * G * S + k + N],
                            start=(n == 0), stop=(n == 5))
                        n += 1
                # x_next = (conv_b + b_conv) + px   (residual)
                nc.vector.scalar_tensor_tensor(
                    _sv(xn[c2][:, g * G * S + 2: g * G * S + 3],
                        [(S, G), (1, Lp)]),
                    _sv(ps[:, 1:2], [(S, G), (1, Lp)]),
                    vec_t[c2][:, 1:2],
                    _sv(px[c2][:, g * G * S + 2: g * G * S + 3],
                        [(S, G), (1, Lp)]),
                    op0=ALU.add, op1=ALU.add)
        x_t = xn
        Sin = S

    # features = x[:, :, 0] per sample  (final packed layout stride 2)
    for c2 in range(NCH):
        nc.sync.dma_start(out=feats_d.ap()[c2],
                          in_=_sv(x_t[c2][:, 2:3], [(2, BLOC), (1, 1)]))
    smlp.release()
    bigp.release()
    perm.release()
    drp.release()
    psp.release()


def _host_inputs(input_ids, emb_table, w_region, b_region, w_conv, b_conv,
                 g1, be1, g2, be2):
    ids = np.asarray(input_ids)
    emb = np.asarray(emb_table)[ids]                     # [B, L, E]
    emb = emb * (ids != PAD_ID)[..., None].astype(np.float32)
    # -> [NCORES, NCI_E, BLOC, 128, L]
    xe = emb.reshape(NCORES, BLOC, L, NCI_E, 128)
    xe = np.ascontiguousarray(xe.transpose(0, 3, 1, 4, 2), dtype=np.float32)

    def pack_lhsT(w, nci):
        # w: [C, nci*128, 3] -> [128, 3*nci*NCH*128]; tile (k,ci,c2) is
        # lhsT[p_in, p_out] = w[c2*128+p_out, ci*128+p_in, k]
        out = np.empty((128, 3 * nci * NCH * 128), np.float32)
        i = 0
        for k in range(3):
            for ci in range(nci):
                for c2 in range(NCH):
                    out[:, i * 128:(i + 1) * 128] = \
                        w[c2 * 128:(c2 + 1) * 128,
                          ci * 128:(ci + 1) * 128, k].T
                    i += 1
        return out

    wr = pack_lhsT(np.asarray(w_region), NCI_E)
    wc = pack_lhsT(np.asarray(w_conv), NCH)
    vec = np.stack([np.asarray(v, np.float32).reshape(NCH, 128)
                    for v in (b_region, b_conv, g1, be1, g2, be2)],
                   axis=-1)                              # [NCH, 128, 6]
    # vec2: [128, 8] cols = g1(c2=0), g1(1), be1(0), be1(1), g2.., be2..
    vec2 = np.stack([np.asarray(v, np.float32).reshape(NCH, 128)[c2]
                     for v in (g1, be1, g2, be2) for c2 in range(NCH)],
                    axis=-1)                             # [128, 8]
    return xe, wr, wc, np.ascontiguousarray(vec), np.ascontiguousarray(vec2)


def _head_loss(features, groups, labels, w_heads, b_heads):
    groups = np.asarray(groups)
    labels = np.asarray(labels)
    w_heads = np.asarray(w_heads)
    b_heads = np.asarray(b_heads)
    logits_all = np.einsum('bd,kdc->bkc', features, w_heads) + b_heads[None]
    idx = np.clip(np.argmax(groups, axis=-1), 0, 4)
    logits = logits_all[np.arange(len(idx)), idx]
    m = logits.max(axis=-1, keepdims=True)
    z = logits - m
    logp = z - np.log(np.exp(z).sum(axis=-1, keepdims=True))
    return np.array(-np.mean(logp[np.arange(len(labels)), labels]),
                    dtype=np.float32)


def _features_from_results(results):
    feats = np.empty((B, C), np.float32)
    for c in range(NCORES):
        f = results[c]["feats"]                          # [NCH, 128, BLOC]
        feats[c * BLOC:(c + 1) * BLOC] = \
            f.transpose(2, 0, 1).reshape(BLOC, C)
    return feats


def kernel(input_ids, groups, labels, emb_table, w_region, b_region,
           w_conv, b_conv, g1, be1, g2, be2, w_heads, b_heads,
           _run_kwargs=None):
    if "nc" not in _CACHE:
        _CACHE["nc"] = _build()
    nc = _CACHE["nc"]

    xe, wr, wc, vec, vec2 = _host_inputs(
        input_ids, emb_table, w_region, b_region, w_conv, b_conv,
        g1, be1, g2, be2)
    in_maps = [{"xe": np.ascontiguousarray(xe[c]), "wr": wr, "wc": wc,
                "vec": vec, "vec2": vec2} for c in range(NCORES)]
    res = bass_utils.run_bass_kernel_spmd(
        nc, in_maps, core_ids=list(range(NCORES)), **(_run_kwargs or {}))
    _CACHE["last_result"] = res
    feats = _features_from_results(res.results)
    _CACHE["features"] = feats
    return _head_loss(feats, groups, labels, w_heads, b_heads)

